# revision 8
# baseline (speedup 1.0000x reference)
"""Trainium2 Bass kernel for DetectionLayer (refine + per-class NMS).

Contract: kernel(rois, probs, deltas) with FULL inputs
  rois   [16, 4096, 4]   f32
  probs  [16, 4096, 81]  f32
  deltas [16, 4096, 81, 4] f32
returns [16, 100, 6] f32 detections, matching the jax reference.

Sharding: pure data parallel — 2 images per core across 8 NeuronCores.

Fast path (always): stream probs for both images (free-dim chunks split
across the sync + scalar HWDGE rings so all 16 SDMA engines stay busy),
per-roi class max overlapped chunk-by-chunk, one total count of rois
passing min-confidence, one values_load.  The detection buffer is
zero-written to DRAM early, hidden under the probs stream.

Slow path (tc.If(total > 0), skipped entirely for inputs where nothing
passes the 0.7 gate): deltas load, argmax-class delta select, box
refine, per-image NMS loops, then a predicated re-write of the real
detections over the zeros.
"""

import numpy as np

import concourse.bacc as bacc
import concourse.bass as bass
import concourse.bass_isa as bass_isa
import concourse.mybir as mybir
from concourse.expressions import smin
from concourse.tile import TileContext

B = 16              # full batch
NCORES = 8
BPC = B // NCORES   # images per core
N = 4096            # rois per image
C = 81              # classes
K = 100             # detection_max_instances
P = 128             # SBUF partitions
NP = N // P         # rois per partition (32)
NH = NP // 2        # rois per partition per DMA chunk (16)
NEG = -1e9
MIN_CONF = 0.7
NMS_T = 0.3
F32 = mybir.dt.float32
I32 = mybir.dt.int32


def _build_image_slow(nc, tc, pools, img, rois_t, deltas_t, state, det):
    """Refine + NMS for one image. Runs inside the total>0 guard only."""
    cpool, big, sm, pp = pools
    pt = state["pt"][img]
    crev = state["crev"]
    negs = state["negs"]

    rois_ap = rois_t[img].rearrange("(p n) k -> p n k", p=P)          # [128,32,4]
    deltas_ap = deltas_t[img].rearrange("(p n) c k -> p n c k", p=P)

    # deltas: two free-dim chunks on the two HWDGE rings (all 16 SDMA engines)
    dt_ = big.tile([P, NP, C, 4], F32, tag=f"deltas{img}")
    nc.sync.dma_start(out=dt_[:, 0:NH], in_=deltas_ap[:, 0:NH])
    nc.scalar.dma_start(out=dt_[:, NH:NP], in_=deltas_ap[:, NH:NP])
    rt = sm.tile([P, NP, 4], F32, tag=f"rois{img}")
    nc.sync.dma_start(out=rt, in_=rois_ap)

    # per-roi class max + exact pass mask (fast path no longer computes these)
    scores = sm.tile([P, NP], F32, tag=f"scores{img}")
    nc.vector.reduce_max(scores, pt, axis=mybir.AxisListType.X)
    ge = sm.tile([P, NP], F32, tag=f"ge{img}")
    c2 = sm.tile([P, 1], F32, tag=f"c2{img}")
    nc.vector.tensor_scalar(
        out=ge, in0=scores, scalar1=MIN_CONF, scalar2=None,
        op0=mybir.AluOpType.is_ge, op1=mybir.AluOpType.add,
        accum_out=c2,
    )

    # one-hot mask of argmax class: M = (probs == score), in place over probs
    m = pt
    nc.vector.tensor_tensor(
        m, pt, scores.unsqueeze(2).to_broadcast([P, NP, C]),
        op=mybir.AluOpType.is_equal,
    )

    # select argmax-class delta: deltas *= M (bcast over k), sum over c
    d_perm = dt_.rearrange("p n c k -> p n k c")
    nc.vector.tensor_tensor(
        d_perm, d_perm, m.unsqueeze(2).to_broadcast([P, NP, 4, C]),
        op=mybir.AluOpType.mult,
    )
    dsel = sm.tile([P, NP, 4], F32, tag=f"dsel{img}")
    nc.vector.reduce_sum(dsel, d_perm, axis=mybir.AxisListType.X)

    # class id = 80 - max((80-c) * M)  (ties -> smallest c, like argmax)
    nc.vector.tensor_tensor(m, m, crev, op=mybir.AluOpType.mult)
    cid = sm.tile([P, NP], F32, tag=f"cid{img}")
    nc.vector.reduce_max(cid, m, axis=mybir.AxisListType.X)
    nc.vector.tensor_scalar(
        out=cid, in0=cid, scalar1=-1.0, scalar2=float(C - 1),
        op0=mybir.AluOpType.mult, op1=mybir.AluOpType.add,
    )

    # bbox_std scaling (match reference op order exactly)
    nc.vector.tensor_scalar_mul(dsel[:, :, 0:2], dsel[:, :, 0:2], 0.1)
    nc.vector.tensor_scalar_mul(dsel[:, :, 2:4], dsel[:, :, 2:4], 0.2)

    # ---- apply deltas + clip (mirrors _apply_deltas fp32 op order) ----
    h = sm.tile([P, NP], F32, tag=f"h{img}")
    w = sm.tile([P, NP], F32, tag=f"w{img}")
    nc.vector.tensor_sub(h, rt[:, :, 2], rt[:, :, 0])
    nc.vector.tensor_sub(w, rt[:, :, 3], rt[:, :, 1])
    t1 = sm.tile([P, NP], F32, tag=f"t1{img}")
    t2 = sm.tile([P, NP], F32, tag=f"t2{img}")
    cy = sm.tile([P, NP], F32, tag=f"cy{img}")
    cx = sm.tile([P, NP], F32, tag=f"cx{img}")
    # cy = y1 + 0.5*h + dy*h
    nc.vector.tensor_scalar_mul(t1, h, 0.5)
    nc.vector.tensor_add(t2, rt[:, :, 0], t1)
    nc.vector.tensor_mul(t1, dsel[:, :, 0], h)
    nc.vector.tensor_add(cy, t2, t1)
    # cx = x1 + 0.5*w + dx*w
    nc.vector.tensor_scalar_mul(t1, w, 0.5)
    nc.vector.tensor_add(t2, rt[:, :, 1], t1)
    nc.vector.tensor_mul(t1, dsel[:, :, 1], w)
    nc.vector.tensor_add(cx, t2, t1)
    # h *= exp(dh); w *= exp(dw)
    e = sm.tile([P, NP], F32, tag=f"e{img}")
    nc.scalar.activation(e, dsel[:, :, 2], mybir.ActivationFunctionType.Exp)
    nc.vector.tensor_mul(h, h, e)
    nc.scalar.activation(e, dsel[:, :, 3], mybir.ActivationFunctionType.Exp)
    nc.vector.tensor_mul(w, w, e)

    ref = sm.tile([P, NP, 4], F32, tag=f"ref{img}")
    nc.vector.tensor_scalar_mul(t1, h, 0.5)
    nc.vector.tensor_sub(ref[:, :, 0], cy, t1)
    nc.vector.tensor_add(ref[:, :, 2], cy, t1)
    nc.vector.tensor_scalar_mul(t2, w, 0.5)
    nc.vector.tensor_sub(ref[:, :, 1], cx, t2)
    nc.vector.tensor_add(ref[:, :, 3], cx, t2)
    nc.vector.tensor_scalar(
        out=ref, in0=ref, scalar1=0.0, scalar2=1.0,
        op0=mybir.AluOpType.max, op1=mybir.AluOpType.min,
    )

    # ---- NMS state ----
    sc = sm.tile([P, NP], F32, tag=f"sc{img}")
    ob = sm.tile([P, NP, 4], F32, tag=f"ob{img}")
    ar = sm.tile([P, NP], F32, tag=f"ar{img}")
    cat = sm.tile([P, NP, 6], F32, tag=f"cat{img}")
    mr = sm.tile([P, 8], F32, tag=f"mr{img}")

    # valid = (cid > 0) & (score >= MIN_CONF); sc0 = valid ? score : NEG
    vf = sm.tile([P, NP], F32, tag=f"vf{img}")
    nc.vector.tensor_single_scalar(vf, cid, 0.5, op=mybir.AluOpType.is_ge)
    v = sm.tile([P, NP], mybir.dt.uint8, tag=f"v{img}")
    nc.vector.tensor_mul(v, vf, ge)
    nc.vector.tensor_copy(sc, negs)
    nc.vector.copy_predicated(sc, v, scores)

    # offset boxes = ref + 2*cid, per-class NMS trick
    nc.vector.scalar_tensor_tensor(
        out=ob, in0=cid.unsqueeze(2).to_broadcast([P, NP, 4]), scalar=2.0,
        in1=ref, op0=mybir.AluOpType.mult, op1=mybir.AluOpType.add,
    )
    # areas of offset boxes
    ar2 = sm.tile([P, NP, 2], F32, tag=f"ar2{img}")
    nc.vector.tensor_sub(ar2, ob[:, :, 2:4], ob[:, :, 0:2])
    nc.vector.tensor_mul(ar, ar2[:, :, 0], ar2[:, :, 1])
    # cat = [ref(4), cid, score] for one-shot row extraction
    nc.vector.tensor_copy(cat[:, :, 0:4], ref)
    nc.vector.tensor_copy(cat[:, :, 4], cid)
    nc.vector.tensor_copy(cat[:, :, 5], scores)
    nc.vector.memset(mr, NEG)

    # per-image count -> NMS trip bound (inside the guard)
    c2p = pp.tile([1, 1], F32, tag=f"c2p{img}")
    nc.tensor.matmul(c2p, state["ones"], c2, start=True, stop=True)
    c2i = sm.tile([1, 1], I32, tag=f"c2i{img}")
    nc.vector.tensor_copy(c2i, c2p)
    rv = nc.values_load(c2i, min_val=0, max_val=N,
                        skip_runtime_bounds_check=True)

    # ---- NMS loop: T = min(100, count) iterations ----
    dbase = img * K * 6
    with tc.For_i(0, smin(rv, K), name=f"nms{img}") as i:
        pm = sm.tile([P, 1], F32, tag=f"pm{img}")
        nc.vector.reduce_max(pm, sc, axis=mybir.AxisListType.X)
        gm = sm.tile([P, 1], F32, tag=f"gm{img}")
        nc.gpsimd.partition_all_reduce(gm, pm, channels=P,
                                       reduce_op=bass_isa.ReduceOp.max)
        # mask of selected candidate
        msk = sm.tile([P, NP], F32, tag=f"msk{img}")
        nc.vector.tensor_tensor(msk, sc, gm.to_broadcast([P, NP]),
                                op=mybir.AluOpType.is_equal)
        # extract its [ref, cid, score] row via masked sum
        mb6 = sm.tile([P, NP, 6], F32, tag=f"mb6{img}")
        nc.vector.tensor_tensor(
            mb6, cat, msk.unsqueeze(2).to_broadcast([P, NP, 6]),
            op=mybir.AluOpType.mult,
        )
        r6p = sm.tile([P, 6], F32, tag=f"r6p{img}")
        nc.vector.reduce_sum(r6p, mb6.rearrange("p n k -> p k n"),
                             axis=mybir.AxisListType.X)
        r6 = sm.tile([P, 6], F32, tag=f"r6{img}")
        nc.gpsimd.partition_all_reduce(r6, r6p, channels=P,
                                       reduce_op=bass_isa.ReduceOp.add)
        # zero the row when scores are exhausted (gm == NEG)
        okm = sm.tile([P, 1], F32, tag=f"okm{img}")
        nc.vector.tensor_single_scalar(okm, gm, NEG * 0.5,
                                       op=mybir.AluOpType.is_gt)
        nc.vector.tensor_mul(r6, r6, okm.to_broadcast([P, 6]))
        nc.vector.tensor_copy(det[0:1, bass.ds(i * 6 + dbase, 6)], r6[0:1, :])

        # selected offset box, replicated on all partitions
        sb = sm.tile([P, 4], F32, tag=f"sb{img}")
        nc.vector.scalar_tensor_tensor(
            out=sb, in0=r6[:, 4:5].to_broadcast([P, 4]), scalar=2.0,
            in1=r6[:, 0:4], op0=mybir.AluOpType.mult, op1=mybir.AluOpType.add,
        )
        # IoU(selected, all) on offset boxes
        mx = sm.tile([P, NP, 2], F32, tag=f"mx{img}")
        nc.vector.tensor_tensor(
            mx, ob[:, :, 0:2], sb[:, 0:2].unsqueeze(1).to_broadcast([P, NP, 2]),
            op=mybir.AluOpType.max,
        )
        mn = sm.tile([P, NP, 2], F32, tag=f"mn{img}")
        nc.vector.tensor_tensor(
            mn, ob[:, :, 2:4], sb[:, 2:4].unsqueeze(1).to_broadcast([P, NP, 2]),
            op=mybir.AluOpType.min,
        )
        nc.vector.tensor_sub(mn, mn, mx)
        nc.vector.tensor_scalar_max(mn, mn, 0.0)
        inter = sm.tile([P, NP], F32, tag=f"inter{img}")
        nc.vector.tensor_mul(inter, mn[:, :, 0], mn[:, :, 1])
        aa2 = sm.tile([P, 2], F32, tag=f"aa2{img}")
        nc.vector.tensor_sub(aa2, sb[:, 2:4], sb[:, 0:2])
        aa = sm.tile([P, 1], F32, tag=f"aa{img}")
        nc.vector.tensor_mul(aa, aa2[:, 0:1], aa2[:, 1:2])
        # suppress iff 0.3 * union < inter  (union = area_sel + areas - inter)
        u = sm.tile([P, NP], F32, tag=f"u{img}")
        nc.vector.scalar_tensor_tensor(
            out=u, in0=ar, scalar=aa[:, 0:1], in1=inter,
            op0=mybir.AluOpType.add, op1=mybir.AluOpType.subtract,
        )
        sup = sm.tile([P, NP], mybir.dt.uint8, tag=f"sup{img}")
        nc.vector.scalar_tensor_tensor(
            out=sup, in0=u, scalar=NMS_T, in1=inter,
            op0=mybir.AluOpType.mult, op1=mybir.AluOpType.is_lt,
        )
        nc.vector.copy_predicated(sc, sup, negs)
        # kill the selected entry itself (covers zero-area self-IoU)
        nc.vector.tensor_copy(mr[:, 0:1], gm)
        nc.vector.match_replace(out=sc, in_to_replace=mr, in_values=sc,
                                imm_value=NEG)


def build_nc():
    nc = bacc.Bacc("TRN2", target_bir_lowering=False)
    rois_t = nc.dram_tensor("rois", [BPC, N, 4], F32, kind="ExternalInput")
    probs_t = nc.dram_tensor("probs", [BPC, N, C], F32, kind="ExternalInput")
    deltas_t = nc.dram_tensor("deltas", [BPC, N, C, 4], F32, kind="ExternalInput")
    out_t = nc.dram_tensor("out", [BPC, K, 6], F32, kind="ExternalOutput")
    out_flat = out_t.rearrange("b k s -> (b k s)").unsqueeze(0)  # [1, 1200]

    with TileContext(nc) as tc:
        with (
            tc.tile_pool(name="const", bufs=1) as cpool,
            tc.tile_pool(name="big", bufs=1) as big,
            tc.tile_pool(name="small", bufs=1) as sm,
            tc.tile_pool(name="psum", bufs=1, space="PSUM") as pp,
        ):
            pools = (cpool, big, sm, pp)
            state = {}

            # probs for both images: 4 free-dim chunks each (8 rois per
            # partition, 2592B descriptors), spread over the 3 DMA rings
            # (sync + scalar HWDGE, gpsimd SWDGE) so the whole stream runs
            # at the HBM rate.  img0 chunks are first in every ring's FIFO
            # so its gate accumulation starts earliest.
            NQ = NP // 4  # rois per partition per chunk (8)
            pt = []
            for img in range(BPC):
                pti = big.tile([P, NP, C], F32, tag=f"probs{img}",
                               name=f"probs{img}")
                pt.append(pti)
            state["pt"] = pt
            ap = [probs_t[img].rearrange("(p n) c -> p n c", p=P)
                  for img in range(BPC)]

            def chunk(img, c):
                sl = slice(c * NQ, (c + 1) * NQ)
                return pt[img][:, sl], ap[img][:, sl]

            ring = {0: nc.sync, 1: nc.scalar, 2: nc.gpsimd}
            # (ring, img, chunk) issue order: img0 first on each ring
            plan = [(0, 0, 0), (1, 0, 1), (2, 0, 2),
                    (0, 0, 3), (1, 1, 0), (2, 1, 1),
                    (0, 1, 2), (1, 1, 3)]
            for r, img, c in plan:
                dst, src = chunk(img, c)
                ring[r].dma_start(out=dst, in_=src)

            # zero detections; early zero-write lands under the probs stream
            # (same scalar ring as the in-guard real write => FIFO ordered)
            det = sm.tile([1, BPC * K * 6], F32, tag="det")
            nc.gpsimd.memset(det, 0.0)
            nc.scalar.dma_start(out=out_flat, in_=det)

            ones = cpool.tile([P, 1], F32, tag="ones")
            nc.gpsimd.memset(ones, 1.0)
            state["ones"] = ones
            nbias = cpool.tile([P, 1], F32, tag="nbias")
            nc.gpsimd.memset(nbias, -690000.0)

            # gate accumulation, chunk by chunk as the stream lands:
            #   vector: exact count of probs >= 0.7 for img0
            #   scalar: sum of Relu(1e6*p - 0.69e6) for img1 — exactly zero
            #     when no prob clears 0.69, strictly positive for any prob
            #     >= 0.7 (conservative over [0.69, 0.7), still exact-safe:
            #     the guard recomputes exact per-image counts)
            acc8 = sm.tile([P, 8], F32, tag="acc8")
            scrv = sm.tile([P, NQ, C], F32, tag="scrv")
            scrs = sm.tile([P, NQ, C], F32, tag="scrs")
            for c in range(4):
                src0, _ = chunk(0, c)
                nc.vector.tensor_scalar(
                    out=scrv, in0=src0, scalar1=MIN_CONF, scalar2=None,
                    op0=mybir.AluOpType.is_ge, op1=mybir.AluOpType.add,
                    accum_out=acc8[:, c : c + 1],
                )
                src1, _ = chunk(1, c)
                nc.scalar.activation(
                    scrs, src1, mybir.ActivationFunctionType.Relu,
                    bias=nbias, scale=1e6,
                    accum_out=acc8[:, 4 + c : 5 + c],
                )

            # total gate value: >0 iff any roi might pass min-confidence
            acc1 = sm.tile([P, 1], F32, tag="acc1")
            nc.vector.reduce_sum(acc1, acc8, axis=mybir.AxisListType.X)
            totp = pp.tile([1, 1], F32, tag="totp")
            nc.tensor.matmul(totp, ones, acc1, start=True, stop=True)
            toti = sm.tile([1, 1], I32, tag="toti")
            nc.vector.tensor_copy(toti, totp)
            tot = nc.values_load(toti, min_val=0, max_val=2**30,
                                 skip_runtime_bounds_check=True)

            with tc.If(tot > 0):
                # constants used only by refine/NMS
                crev = cpool.tile([P, NP, C], F32, tag="crev")
                nc.gpsimd.iota(crev, pattern=[[0, NP], [-1, C]], base=C - 1,
                               channel_multiplier=0,
                               allow_small_or_imprecise_dtypes=True)
                negs = cpool.tile([P, NP], F32, tag="negs")
                nc.gpsimd.memset(negs, NEG)
                state["crev"] = crev
                state["negs"] = negs
                for img in range(BPC):
                    _build_image_slow(nc, tc, pools, img, rois_t, deltas_t,
                                      state, det)
                # real detections overwrite the zeros; same scalar ring as
                # the zero-write => FIFO-ordered at the destination
                nc.scalar.dma_start(out=out_flat, in_=det)
    nc.compile()
    return nc


LAST_RESULTS = None  # BassKernelResults of the most recent kernel() call


def kernel(rois, probs, deltas):
    global LAST_RESULTS
    from concourse import bass_utils

    nc = build_nc()
    in_maps = []
    for c in range(NCORES):
        sl = slice(c * BPC, (c + 1) * BPC)
        in_maps.append({
            "rois": np.ascontiguousarray(rois[sl], dtype=np.float32),
            "probs": np.ascontiguousarray(probs[sl], dtype=np.float32),
            "deltas": np.ascontiguousarray(deltas[sl], dtype=np.float32),
        })
    res = bass_utils.run_bass_kernel_spmd(nc, in_maps, core_ids=list(range(NCORES)))
    LAST_RESULTS = res
    return np.concatenate([r["out"] for r in res.results], axis=0)


if __name__ == "__main__":
    rng = np.random.default_rng(0)
    out = kernel(
        rng.random((B, N, 4), np.float32),
        rng.random((B, N, C), np.float32),
        rng.standard_normal((B, N, C, 4)).astype(np.float32),
    )
    print(out.shape, np.abs(out).max())


# revision 9
# speedup vs baseline: 1.0178x; 1.0178x over previous
"""Trainium2 Bass kernel for DetectionLayer (refine + per-class NMS).

Contract: kernel(rois, probs, deltas) with FULL inputs
  rois   [16, 4096, 4]   f32
  probs  [16, 4096, 81]  f32
  deltas [16, 4096, 81, 4] f32
returns [16, 100, 6] f32 detections, matching the jax reference.

Sharding: pure data parallel — 2 images per core across 8 NeuronCores.

Fast path (always): stream probs for both images (free-dim chunks split
across the sync + scalar HWDGE rings so all 16 SDMA engines stay busy),
per-roi class max overlapped chunk-by-chunk, one total count of rois
passing min-confidence, one values_load.  The detection buffer is
zero-written to DRAM early, hidden under the probs stream.

Slow path (tc.If(total > 0), skipped entirely for inputs where nothing
passes the 0.7 gate): deltas load, argmax-class delta select, box
refine, per-image NMS loops, then a predicated re-write of the real
detections over the zeros.
"""

import numpy as np

import concourse.bacc as bacc
import concourse.bass as bass
import concourse.bass_isa as bass_isa
import concourse.mybir as mybir
from concourse.expressions import smin
from concourse.tile import TileContext

B = 16              # full batch
NCORES = 8
BPC = B // NCORES   # images per core
N = 4096            # rois per image
C = 81              # classes
K = 100             # detection_max_instances
P = 128             # SBUF partitions
NP = N // P         # rois per partition (32)
NH = NP // 2        # rois per partition per DMA chunk (16)
NEG = -1e9
MIN_CONF = 0.7
NMS_T = 0.3
F32 = mybir.dt.float32
I32 = mybir.dt.int32


def _build_image_slow(nc, tc, pools, img, rois_t, deltas_t, state, det):
    """Refine + NMS for one image. Runs inside the total>0 guard only."""
    cpool, big, sm, pp = pools
    pt = state["pt"][img]
    crev = state["crev"]
    negs = state["negs"]

    rois_ap = rois_t[img].rearrange("(p n) k -> p n k", p=P)          # [128,32,4]
    deltas_ap = deltas_t[img].rearrange("(p n) c k -> p n c k", p=P)

    # deltas: two free-dim chunks on the two HWDGE rings (all 16 SDMA engines)
    dt_ = big.tile([P, NP, C, 4], F32, tag=f"deltas{img}")
    nc.sync.dma_start(out=dt_[:, 0:NH], in_=deltas_ap[:, 0:NH])
    nc.scalar.dma_start(out=dt_[:, NH:NP], in_=deltas_ap[:, NH:NP])
    rt = sm.tile([P, NP, 4], F32, tag=f"rois{img}")
    nc.sync.dma_start(out=rt, in_=rois_ap)

    # per-roi class max + exact pass mask (fast path no longer computes these)
    scores = sm.tile([P, NP], F32, tag=f"scores{img}")
    nc.vector.reduce_max(scores, pt, axis=mybir.AxisListType.X)
    ge = sm.tile([P, NP], F32, tag=f"ge{img}")
    c2 = sm.tile([P, 1], F32, tag=f"c2{img}")
    nc.vector.tensor_scalar(
        out=ge, in0=scores, scalar1=MIN_CONF, scalar2=None,
        op0=mybir.AluOpType.is_ge, op1=mybir.AluOpType.add,
        accum_out=c2,
    )

    # one-hot mask of argmax class: M = (probs == score), in place over probs
    m = pt
    nc.vector.tensor_tensor(
        m, pt, scores.unsqueeze(2).to_broadcast([P, NP, C]),
        op=mybir.AluOpType.is_equal,
    )

    # select argmax-class delta: deltas *= M (bcast over k), sum over c
    d_perm = dt_.rearrange("p n c k -> p n k c")
    nc.vector.tensor_tensor(
        d_perm, d_perm, m.unsqueeze(2).to_broadcast([P, NP, 4, C]),
        op=mybir.AluOpType.mult,
    )
    dsel = sm.tile([P, NP, 4], F32, tag=f"dsel{img}")
    nc.vector.reduce_sum(dsel, d_perm, axis=mybir.AxisListType.X)

    # class id = 80 - max((80-c) * M)  (ties -> smallest c, like argmax)
    nc.vector.tensor_tensor(m, m, crev, op=mybir.AluOpType.mult)
    cid = sm.tile([P, NP], F32, tag=f"cid{img}")
    nc.vector.reduce_max(cid, m, axis=mybir.AxisListType.X)
    nc.vector.tensor_scalar(
        out=cid, in0=cid, scalar1=-1.0, scalar2=float(C - 1),
        op0=mybir.AluOpType.mult, op1=mybir.AluOpType.add,
    )

    # bbox_std scaling (match reference op order exactly)
    nc.vector.tensor_scalar_mul(dsel[:, :, 0:2], dsel[:, :, 0:2], 0.1)
    nc.vector.tensor_scalar_mul(dsel[:, :, 2:4], dsel[:, :, 2:4], 0.2)

    # ---- apply deltas + clip (mirrors _apply_deltas fp32 op order) ----
    h = sm.tile([P, NP], F32, tag=f"h{img}")
    w = sm.tile([P, NP], F32, tag=f"w{img}")
    nc.vector.tensor_sub(h, rt[:, :, 2], rt[:, :, 0])
    nc.vector.tensor_sub(w, rt[:, :, 3], rt[:, :, 1])
    t1 = sm.tile([P, NP], F32, tag=f"t1{img}")
    t2 = sm.tile([P, NP], F32, tag=f"t2{img}")
    cy = sm.tile([P, NP], F32, tag=f"cy{img}")
    cx = sm.tile([P, NP], F32, tag=f"cx{img}")
    # cy = y1 + 0.5*h + dy*h
    nc.vector.tensor_scalar_mul(t1, h, 0.5)
    nc.vector.tensor_add(t2, rt[:, :, 0], t1)
    nc.vector.tensor_mul(t1, dsel[:, :, 0], h)
    nc.vector.tensor_add(cy, t2, t1)
    # cx = x1 + 0.5*w + dx*w
    nc.vector.tensor_scalar_mul(t1, w, 0.5)
    nc.vector.tensor_add(t2, rt[:, :, 1], t1)
    nc.vector.tensor_mul(t1, dsel[:, :, 1], w)
    nc.vector.tensor_add(cx, t2, t1)
    # h *= exp(dh); w *= exp(dw)
    e = sm.tile([P, NP], F32, tag=f"e{img}")
    nc.scalar.activation(e, dsel[:, :, 2], mybir.ActivationFunctionType.Exp)
    nc.vector.tensor_mul(h, h, e)
    nc.scalar.activation(e, dsel[:, :, 3], mybir.ActivationFunctionType.Exp)
    nc.vector.tensor_mul(w, w, e)

    ref = sm.tile([P, NP, 4], F32, tag=f"ref{img}")
    nc.vector.tensor_scalar_mul(t1, h, 0.5)
    nc.vector.tensor_sub(ref[:, :, 0], cy, t1)
    nc.vector.tensor_add(ref[:, :, 2], cy, t1)
    nc.vector.tensor_scalar_mul(t2, w, 0.5)
    nc.vector.tensor_sub(ref[:, :, 1], cx, t2)
    nc.vector.tensor_add(ref[:, :, 3], cx, t2)
    nc.vector.tensor_scalar(
        out=ref, in0=ref, scalar1=0.0, scalar2=1.0,
        op0=mybir.AluOpType.max, op1=mybir.AluOpType.min,
    )

    # ---- NMS state ----
    sc = sm.tile([P, NP], F32, tag=f"sc{img}")
    ob = sm.tile([P, NP, 4], F32, tag=f"ob{img}")
    ar = sm.tile([P, NP], F32, tag=f"ar{img}")
    cat = sm.tile([P, NP, 6], F32, tag=f"cat{img}")
    mr = sm.tile([P, 8], F32, tag=f"mr{img}")

    # valid = (cid > 0) & (score >= MIN_CONF); sc0 = valid ? score : NEG
    vf = sm.tile([P, NP], F32, tag=f"vf{img}")
    nc.vector.tensor_single_scalar(vf, cid, 0.5, op=mybir.AluOpType.is_ge)
    v = sm.tile([P, NP], mybir.dt.uint8, tag=f"v{img}")
    nc.vector.tensor_mul(v, vf, ge)
    nc.vector.tensor_copy(sc, negs)
    nc.vector.copy_predicated(sc, v, scores)

    # offset boxes = ref + 2*cid, per-class NMS trick
    nc.vector.scalar_tensor_tensor(
        out=ob, in0=cid.unsqueeze(2).to_broadcast([P, NP, 4]), scalar=2.0,
        in1=ref, op0=mybir.AluOpType.mult, op1=mybir.AluOpType.add,
    )
    # areas of offset boxes
    ar2 = sm.tile([P, NP, 2], F32, tag=f"ar2{img}")
    nc.vector.tensor_sub(ar2, ob[:, :, 2:4], ob[:, :, 0:2])
    nc.vector.tensor_mul(ar, ar2[:, :, 0], ar2[:, :, 1])
    # cat = [ref(4), cid, score] for one-shot row extraction
    nc.vector.tensor_copy(cat[:, :, 0:4], ref)
    nc.vector.tensor_copy(cat[:, :, 4], cid)
    nc.vector.tensor_copy(cat[:, :, 5], scores)
    nc.vector.memset(mr, NEG)

    # per-image count -> NMS trip bound (inside the guard)
    c2p = pp.tile([1, 1], F32, tag=f"c2p{img}")
    nc.tensor.matmul(c2p, state["ones"], c2, start=True, stop=True)
    c2i = sm.tile([1, 1], I32, tag=f"c2i{img}")
    nc.vector.tensor_copy(c2i, c2p)
    rv = nc.values_load(c2i, min_val=0, max_val=N,
                        skip_runtime_bounds_check=True)

    # ---- NMS loop: T = min(100, count) iterations ----
    dbase = img * K * 6
    with tc.For_i(0, smin(rv, K), name=f"nms{img}") as i:
        pm = sm.tile([P, 1], F32, tag=f"pm{img}")
        nc.vector.reduce_max(pm, sc, axis=mybir.AxisListType.X)
        gm = sm.tile([P, 1], F32, tag=f"gm{img}")
        nc.gpsimd.partition_all_reduce(gm, pm, channels=P,
                                       reduce_op=bass_isa.ReduceOp.max)
        # mask of selected candidate
        msk = sm.tile([P, NP], F32, tag=f"msk{img}")
        nc.vector.tensor_tensor(msk, sc, gm.to_broadcast([P, NP]),
                                op=mybir.AluOpType.is_equal)
        # extract its [ref, cid, score] row via masked sum
        mb6 = sm.tile([P, NP, 6], F32, tag=f"mb6{img}")
        nc.vector.tensor_tensor(
            mb6, cat, msk.unsqueeze(2).to_broadcast([P, NP, 6]),
            op=mybir.AluOpType.mult,
        )
        r6p = sm.tile([P, 6], F32, tag=f"r6p{img}")
        nc.vector.reduce_sum(r6p, mb6.rearrange("p n k -> p k n"),
                             axis=mybir.AxisListType.X)
        r6 = sm.tile([P, 6], F32, tag=f"r6{img}")
        nc.gpsimd.partition_all_reduce(r6, r6p, channels=P,
                                       reduce_op=bass_isa.ReduceOp.add)
        # zero the row when scores are exhausted (gm == NEG)
        okm = sm.tile([P, 1], F32, tag=f"okm{img}")
        nc.vector.tensor_single_scalar(okm, gm, NEG * 0.5,
                                       op=mybir.AluOpType.is_gt)
        nc.vector.tensor_mul(r6, r6, okm.to_broadcast([P, 6]))
        nc.vector.tensor_copy(det[0:1, bass.ds(i * 6 + dbase, 6)], r6[0:1, :])

        # selected offset box, replicated on all partitions
        sb = sm.tile([P, 4], F32, tag=f"sb{img}")
        nc.vector.scalar_tensor_tensor(
            out=sb, in0=r6[:, 4:5].to_broadcast([P, 4]), scalar=2.0,
            in1=r6[:, 0:4], op0=mybir.AluOpType.mult, op1=mybir.AluOpType.add,
        )
        # IoU(selected, all) on offset boxes
        mx = sm.tile([P, NP, 2], F32, tag=f"mx{img}")
        nc.vector.tensor_tensor(
            mx, ob[:, :, 0:2], sb[:, 0:2].unsqueeze(1).to_broadcast([P, NP, 2]),
            op=mybir.AluOpType.max,
        )
        mn = sm.tile([P, NP, 2], F32, tag=f"mn{img}")
        nc.vector.tensor_tensor(
            mn, ob[:, :, 2:4], sb[:, 2:4].unsqueeze(1).to_broadcast([P, NP, 2]),
            op=mybir.AluOpType.min,
        )
        nc.vector.tensor_sub(mn, mn, mx)
        nc.vector.tensor_scalar_max(mn, mn, 0.0)
        inter = sm.tile([P, NP], F32, tag=f"inter{img}")
        nc.vector.tensor_mul(inter, mn[:, :, 0], mn[:, :, 1])
        aa2 = sm.tile([P, 2], F32, tag=f"aa2{img}")
        nc.vector.tensor_sub(aa2, sb[:, 2:4], sb[:, 0:2])
        aa = sm.tile([P, 1], F32, tag=f"aa{img}")
        nc.vector.tensor_mul(aa, aa2[:, 0:1], aa2[:, 1:2])
        # suppress iff 0.3 * union < inter  (union = area_sel + areas - inter)
        u = sm.tile([P, NP], F32, tag=f"u{img}")
        nc.vector.scalar_tensor_tensor(
            out=u, in0=ar, scalar=aa[:, 0:1], in1=inter,
            op0=mybir.AluOpType.add, op1=mybir.AluOpType.subtract,
        )
        sup = sm.tile([P, NP], mybir.dt.uint8, tag=f"sup{img}")
        nc.vector.scalar_tensor_tensor(
            out=sup, in0=u, scalar=NMS_T, in1=inter,
            op0=mybir.AluOpType.mult, op1=mybir.AluOpType.is_lt,
        )
        nc.vector.copy_predicated(sc, sup, negs)
        # kill the selected entry itself (covers zero-area self-IoU)
        nc.vector.tensor_copy(mr[:, 0:1], gm)
        nc.vector.match_replace(out=sc, in_to_replace=mr, in_values=sc,
                                imm_value=NEG)


def build_nc():
    nc = bacc.Bacc("TRN2", target_bir_lowering=False)
    rois_t = nc.dram_tensor("rois", [BPC, N, 4], F32, kind="ExternalInput")
    probs_t = nc.dram_tensor("probs", [BPC, N, C], F32, kind="ExternalInput")
    deltas_t = nc.dram_tensor("deltas", [BPC, N, C, 4], F32, kind="ExternalInput")
    out_t = nc.dram_tensor("out", [BPC, K, 6], F32, kind="ExternalOutput")
    out_flat = out_t.rearrange("b k s -> (b k s)").unsqueeze(0)  # [1, 1200]

    with TileContext(nc) as tc:
        with (
            tc.tile_pool(name="const", bufs=1) as cpool,
            tc.tile_pool(name="big", bufs=1) as big,
            tc.tile_pool(name="small", bufs=1) as sm,
            tc.tile_pool(name="psum", bufs=1, space="PSUM") as pp,
        ):
            pools = (cpool, big, sm, pp)
            state = {}

            # probs for both images: 4 free-dim chunks each (8 rois per
            # partition, 2592B descriptors), spread over the 3 DMA rings
            # (sync + scalar HWDGE, gpsimd SWDGE) so the whole stream runs
            # at the HBM rate.  img0 chunks are first in every ring's FIFO
            # so its gate accumulation starts earliest.
            NQ = NP // 4  # rois per partition per chunk (8)
            pt = []
            for img in range(BPC):
                pti = big.tile([P, NP, C], F32, tag=f"probs{img}",
                               name=f"probs{img}")
                pt.append(pti)
            state["pt"] = pt
            ap = [probs_t[img].rearrange("(p n) c -> p n c", p=P)
                  for img in range(BPC)]

            def chunk(img, c):
                sl = slice(c * NQ, (c + 1) * NQ)
                return pt[img][:, sl], ap[img][:, sl]

            ring = {0: nc.sync, 1: nc.scalar, 2: nc.gpsimd}
            # (ring, img, chunk) issue order: img0 first on each ring
            plan = [(0, 0, 0), (1, 0, 1), (2, 0, 2),
                    (0, 0, 3), (1, 1, 0), (2, 1, 1),
                    (0, 1, 2), (1, 1, 3)]
            for r, img, c in plan:
                dst, src = chunk(img, c)
                ring[r].dma_start(out=dst, in_=src)

            # zero detections; early zero-write lands under the probs stream
            # (same scalar ring as the in-guard real write => FIFO ordered)
            det = sm.tile([1, BPC * K * 6], F32, tag="det")
            nc.gpsimd.memset(det, 0.0)
            nc.scalar.dma_start(out=out_flat, in_=det)

            ones = cpool.tile([P, 1], F32, tag="ones")
            nc.gpsimd.memset(ones, 1.0)
            state["ones"] = ones
            nbias = cpool.tile([P, 1], F32, tag="nbias")
            nc.gpsimd.memset(nbias, -690000.0)

            # gate accumulation, chunk by chunk as the stream lands:
            #   vector: exact count of probs >= 0.7 for img0
            #   scalar: sum of Relu(1e6*p - 0.69e6) for img1 — exactly zero
            #     when no prob clears 0.69, strictly positive for any prob
            #     >= 0.7 (conservative over [0.69, 0.7), still exact-safe:
            #     the guard recomputes exact per-image counts)
            # chunk -> accum engine, interleaved by expected landing order so
            # neither engine is stuck with only late-landing chunks (the DVE,
            # faster per chunk, takes the stragglers)
            acc8 = sm.tile([P, 8], F32, tag="acc8")
            scrv = sm.tile([P, NQ, C], F32, tag="scrv")
            scrs = sm.tile([P, NQ, C], F32, tag="scrs")
            vec_chunks = [(0, 0), (0, 2), (1, 0), (1, 2), (1, 3)]
            act_chunks = [(0, 1), (0, 3), (1, 1)]
            for k, (img, c) in enumerate(vec_chunks):
                src, _ = chunk(img, c)
                nc.vector.tensor_scalar(
                    out=scrv, in0=src, scalar1=MIN_CONF, scalar2=None,
                    op0=mybir.AluOpType.is_ge, op1=mybir.AluOpType.add,
                    accum_out=acc8[:, k : k + 1],
                )
            for k, (img, c) in enumerate(act_chunks):
                src, _ = chunk(img, c)
                nc.scalar.activation(
                    scrs, src, mybir.ActivationFunctionType.Relu,
                    bias=nbias, scale=1e6,
                    accum_out=acc8[:, 5 + k : 6 + k],
                )

            # total gate value: >0 iff any roi might pass min-confidence
            acc1 = sm.tile([P, 1], F32, tag="acc1")
            nc.vector.reduce_sum(acc1, acc8, axis=mybir.AxisListType.X)
            totp = pp.tile([1, 1], F32, tag="totp")
            nc.tensor.matmul(totp, ones, acc1, start=True, stop=True)
            toti = sm.tile([1, 1], I32, tag="toti")
            nc.vector.tensor_copy(toti, totp)
            tot = nc.values_load(toti, min_val=0, max_val=2**30,
                                 skip_runtime_bounds_check=True)

            with tc.If(tot > 0):
                # constants used only by refine/NMS
                crev = cpool.tile([P, NP, C], F32, tag="crev")
                nc.gpsimd.iota(crev, pattern=[[0, NP], [-1, C]], base=C - 1,
                               channel_multiplier=0,
                               allow_small_or_imprecise_dtypes=True)
                negs = cpool.tile([P, NP], F32, tag="negs")
                nc.gpsimd.memset(negs, NEG)
                state["crev"] = crev
                state["negs"] = negs
                for img in range(BPC):
                    _build_image_slow(nc, tc, pools, img, rois_t, deltas_t,
                                      state, det)
                # real detections overwrite the zeros; same scalar ring as
                # the zero-write => FIFO-ordered at the destination
                nc.scalar.dma_start(out=out_flat, in_=det)
    nc.compile()
    return nc


LAST_RESULTS = None  # BassKernelResults of the most recent kernel() call


def kernel(rois, probs, deltas):
    global LAST_RESULTS
    from concourse import bass_utils

    nc = build_nc()
    in_maps = []
    for c in range(NCORES):
        sl = slice(c * BPC, (c + 1) * BPC)
        in_maps.append({
            "rois": np.ascontiguousarray(rois[sl], dtype=np.float32),
            "probs": np.ascontiguousarray(probs[sl], dtype=np.float32),
            "deltas": np.ascontiguousarray(deltas[sl], dtype=np.float32),
        })
    res = bass_utils.run_bass_kernel_spmd(nc, in_maps, core_ids=list(range(NCORES)))
    LAST_RESULTS = res
    return np.concatenate([r["out"] for r in res.results], axis=0)


if __name__ == "__main__":
    rng = np.random.default_rng(0)
    out = kernel(
        rng.random((B, N, 4), np.float32),
        rng.random((B, N, C), np.float32),
        rng.standard_normal((B, N, C, 4)).astype(np.float32),
    )
    print(out.shape, np.abs(out).max())


# revision 11
# speedup vs baseline: 1.0303x; 1.0122x over previous
"""Trainium2 Bass kernel for DetectionLayer (refine + per-class NMS).

Contract: kernel(rois, probs, deltas) with FULL inputs
  rois   [16, 4096, 4]   f32
  probs  [16, 4096, 81]  f32
  deltas [16, 4096, 81, 4] f32
returns [16, 100, 6] f32 detections, matching the jax reference.

Sharding: pure data parallel — 2 images per core across 8 NeuronCores.

Fast path (always): stream probs for both images (free-dim chunks split
across the sync + scalar HWDGE rings so all 16 SDMA engines stay busy),
per-roi class max overlapped chunk-by-chunk, one total count of rois
passing min-confidence, one values_load.  The detection buffer is
zero-written to DRAM early, hidden under the probs stream.

Slow path (tc.If(total > 0), skipped entirely for inputs where nothing
passes the 0.7 gate): deltas load, argmax-class delta select, box
refine, per-image NMS loops, then a predicated re-write of the real
detections over the zeros.
"""

import numpy as np

import concourse.bacc as bacc
import concourse.bass as bass
import concourse.bass_isa as bass_isa
import concourse.mybir as mybir
from concourse.expressions import smin
from concourse.tile import TileContext

B = 16              # full batch
NCORES = 8
BPC = B // NCORES   # images per core
N = 4096            # rois per image
C = 81              # classes
K = 100             # detection_max_instances
P = 128             # SBUF partitions
NP = N // P         # rois per partition (32)
NH = NP // 2        # rois per partition per DMA chunk (16)
NEG = -1e9
MIN_CONF = 0.7
NMS_T = 0.3
F32 = mybir.dt.float32
I32 = mybir.dt.int32


def _build_image_slow(nc, tc, pools, img, rois_t, deltas_t, state, det):
    """Refine + NMS for one image. Runs inside the total>0 guard only."""
    cpool, big, sm, pp = pools
    pt = state["pt"][img]
    crev = state["crev"]
    negs = state["negs"]

    rois_ap = rois_t[img].rearrange("(p n) k -> p n k", p=P)          # [128,32,4]
    deltas_ap = deltas_t[img].rearrange("(p n) c k -> p n c k", p=P)

    # deltas: two free-dim chunks on the two HWDGE rings (all 16 SDMA engines)
    dt_ = big.tile([P, NP, C, 4], F32, tag=f"deltas{img}")
    nc.sync.dma_start(out=dt_[:, 0:NH], in_=deltas_ap[:, 0:NH])
    nc.scalar.dma_start(out=dt_[:, NH:NP], in_=deltas_ap[:, NH:NP])
    rt = sm.tile([P, NP, 4], F32, tag=f"rois{img}")
    nc.sync.dma_start(out=rt, in_=rois_ap)

    # per-roi class max + exact pass mask (fast path no longer computes these)
    scores = sm.tile([P, NP], F32, tag=f"scores{img}")
    nc.vector.reduce_max(scores, pt, axis=mybir.AxisListType.X)
    ge = sm.tile([P, NP], F32, tag=f"ge{img}")
    c2 = sm.tile([P, 1], F32, tag=f"c2{img}")
    nc.vector.tensor_scalar(
        out=ge, in0=scores, scalar1=MIN_CONF, scalar2=None,
        op0=mybir.AluOpType.is_ge, op1=mybir.AluOpType.add,
        accum_out=c2,
    )

    # one-hot mask of argmax class: M = (probs == score), in place over probs
    m = pt
    nc.vector.tensor_tensor(
        m, pt, scores.unsqueeze(2).to_broadcast([P, NP, C]),
        op=mybir.AluOpType.is_equal,
    )

    # select argmax-class delta: deltas *= M (bcast over k), sum over c
    d_perm = dt_.rearrange("p n c k -> p n k c")
    nc.vector.tensor_tensor(
        d_perm, d_perm, m.unsqueeze(2).to_broadcast([P, NP, 4, C]),
        op=mybir.AluOpType.mult,
    )
    dsel = sm.tile([P, NP, 4], F32, tag=f"dsel{img}")
    nc.vector.reduce_sum(dsel, d_perm, axis=mybir.AxisListType.X)

    # class id = 80 - max((80-c) * M)  (ties -> smallest c, like argmax)
    nc.vector.tensor_tensor(m, m, crev, op=mybir.AluOpType.mult)
    cid = sm.tile([P, NP], F32, tag=f"cid{img}")
    nc.vector.reduce_max(cid, m, axis=mybir.AxisListType.X)
    nc.vector.tensor_scalar(
        out=cid, in0=cid, scalar1=-1.0, scalar2=float(C - 1),
        op0=mybir.AluOpType.mult, op1=mybir.AluOpType.add,
    )

    # bbox_std scaling (match reference op order exactly)
    nc.vector.tensor_scalar_mul(dsel[:, :, 0:2], dsel[:, :, 0:2], 0.1)
    nc.vector.tensor_scalar_mul(dsel[:, :, 2:4], dsel[:, :, 2:4], 0.2)

    # ---- apply deltas + clip (mirrors _apply_deltas fp32 op order) ----
    h = sm.tile([P, NP], F32, tag=f"h{img}")
    w = sm.tile([P, NP], F32, tag=f"w{img}")
    nc.vector.tensor_sub(h, rt[:, :, 2], rt[:, :, 0])
    nc.vector.tensor_sub(w, rt[:, :, 3], rt[:, :, 1])
    t1 = sm.tile([P, NP], F32, tag=f"t1{img}")
    t2 = sm.tile([P, NP], F32, tag=f"t2{img}")
    cy = sm.tile([P, NP], F32, tag=f"cy{img}")
    cx = sm.tile([P, NP], F32, tag=f"cx{img}")
    # cy = y1 + 0.5*h + dy*h
    nc.vector.tensor_scalar_mul(t1, h, 0.5)
    nc.vector.tensor_add(t2, rt[:, :, 0], t1)
    nc.vector.tensor_mul(t1, dsel[:, :, 0], h)
    nc.vector.tensor_add(cy, t2, t1)
    # cx = x1 + 0.5*w + dx*w
    nc.vector.tensor_scalar_mul(t1, w, 0.5)
    nc.vector.tensor_add(t2, rt[:, :, 1], t1)
    nc.vector.tensor_mul(t1, dsel[:, :, 1], w)
    nc.vector.tensor_add(cx, t2, t1)
    # h *= exp(dh); w *= exp(dw)
    e = sm.tile([P, NP], F32, tag=f"e{img}")
    nc.scalar.activation(e, dsel[:, :, 2], mybir.ActivationFunctionType.Exp)
    nc.vector.tensor_mul(h, h, e)
    nc.scalar.activation(e, dsel[:, :, 3], mybir.ActivationFunctionType.Exp)
    nc.vector.tensor_mul(w, w, e)

    ref = sm.tile([P, NP, 4], F32, tag=f"ref{img}")
    nc.vector.tensor_scalar_mul(t1, h, 0.5)
    nc.vector.tensor_sub(ref[:, :, 0], cy, t1)
    nc.vector.tensor_add(ref[:, :, 2], cy, t1)
    nc.vector.tensor_scalar_mul(t2, w, 0.5)
    nc.vector.tensor_sub(ref[:, :, 1], cx, t2)
    nc.vector.tensor_add(ref[:, :, 3], cx, t2)
    nc.vector.tensor_scalar(
        out=ref, in0=ref, scalar1=0.0, scalar2=1.0,
        op0=mybir.AluOpType.max, op1=mybir.AluOpType.min,
    )

    # ---- NMS state ----
    sc = sm.tile([P, NP], F32, tag=f"sc{img}")
    ob = sm.tile([P, NP, 4], F32, tag=f"ob{img}")
    ar = sm.tile([P, NP], F32, tag=f"ar{img}")
    cat = sm.tile([P, NP, 6], F32, tag=f"cat{img}")
    mr = sm.tile([P, 8], F32, tag=f"mr{img}")

    # valid = (cid > 0) & (score >= MIN_CONF); sc0 = valid ? score : NEG
    vf = sm.tile([P, NP], F32, tag=f"vf{img}")
    nc.vector.tensor_single_scalar(vf, cid, 0.5, op=mybir.AluOpType.is_ge)
    v = sm.tile([P, NP], mybir.dt.uint8, tag=f"v{img}")
    nc.vector.tensor_mul(v, vf, ge)
    nc.vector.tensor_copy(sc, negs)
    nc.vector.copy_predicated(sc, v, scores)

    # offset boxes = ref + 2*cid, per-class NMS trick
    nc.vector.scalar_tensor_tensor(
        out=ob, in0=cid.unsqueeze(2).to_broadcast([P, NP, 4]), scalar=2.0,
        in1=ref, op0=mybir.AluOpType.mult, op1=mybir.AluOpType.add,
    )
    # areas of offset boxes
    ar2 = sm.tile([P, NP, 2], F32, tag=f"ar2{img}")
    nc.vector.tensor_sub(ar2, ob[:, :, 2:4], ob[:, :, 0:2])
    nc.vector.tensor_mul(ar, ar2[:, :, 0], ar2[:, :, 1])
    # cat = [ref(4), cid, score] for one-shot row extraction
    nc.vector.tensor_copy(cat[:, :, 0:4], ref)
    nc.vector.tensor_copy(cat[:, :, 4], cid)
    nc.vector.tensor_copy(cat[:, :, 5], scores)
    nc.vector.memset(mr, NEG)

    # per-image count -> NMS trip bound (inside the guard)
    c2p = pp.tile([1, 1], F32, tag=f"c2p{img}")
    nc.tensor.matmul(c2p, state["ones"], c2, start=True, stop=True)
    c2i = sm.tile([1, 1], I32, tag=f"c2i{img}")
    nc.vector.tensor_copy(c2i, c2p)
    rv = nc.values_load(c2i, min_val=0, max_val=N,
                        skip_runtime_bounds_check=True)

    # ---- NMS loop: T = min(100, count) iterations ----
    dbase = img * K * 6
    with tc.For_i(0, smin(rv, K), name=f"nms{img}") as i:
        pm = sm.tile([P, 1], F32, tag=f"pm{img}")
        nc.vector.reduce_max(pm, sc, axis=mybir.AxisListType.X)
        gm = sm.tile([P, 1], F32, tag=f"gm{img}")
        nc.gpsimd.partition_all_reduce(gm, pm, channels=P,
                                       reduce_op=bass_isa.ReduceOp.max)
        # mask of selected candidate
        msk = sm.tile([P, NP], F32, tag=f"msk{img}")
        nc.vector.tensor_tensor(msk, sc, gm.to_broadcast([P, NP]),
                                op=mybir.AluOpType.is_equal)
        # extract its [ref, cid, score] row via masked sum
        mb6 = sm.tile([P, NP, 6], F32, tag=f"mb6{img}")
        nc.vector.tensor_tensor(
            mb6, cat, msk.unsqueeze(2).to_broadcast([P, NP, 6]),
            op=mybir.AluOpType.mult,
        )
        r6p = sm.tile([P, 6], F32, tag=f"r6p{img}")
        nc.vector.reduce_sum(r6p, mb6.rearrange("p n k -> p k n"),
                             axis=mybir.AxisListType.X)
        r6 = sm.tile([P, 6], F32, tag=f"r6{img}")
        nc.gpsimd.partition_all_reduce(r6, r6p, channels=P,
                                       reduce_op=bass_isa.ReduceOp.add)
        # zero the row when scores are exhausted (gm == NEG)
        okm = sm.tile([P, 1], F32, tag=f"okm{img}")
        nc.vector.tensor_single_scalar(okm, gm, NEG * 0.5,
                                       op=mybir.AluOpType.is_gt)
        nc.vector.tensor_mul(r6, r6, okm.to_broadcast([P, 6]))
        nc.vector.tensor_copy(det[0:1, bass.ds(i * 6 + dbase, 6)], r6[0:1, :])

        # selected offset box, replicated on all partitions
        sb = sm.tile([P, 4], F32, tag=f"sb{img}")
        nc.vector.scalar_tensor_tensor(
            out=sb, in0=r6[:, 4:5].to_broadcast([P, 4]), scalar=2.0,
            in1=r6[:, 0:4], op0=mybir.AluOpType.mult, op1=mybir.AluOpType.add,
        )
        # IoU(selected, all) on offset boxes
        mx = sm.tile([P, NP, 2], F32, tag=f"mx{img}")
        nc.vector.tensor_tensor(
            mx, ob[:, :, 0:2], sb[:, 0:2].unsqueeze(1).to_broadcast([P, NP, 2]),
            op=mybir.AluOpType.max,
        )
        mn = sm.tile([P, NP, 2], F32, tag=f"mn{img}")
        nc.vector.tensor_tensor(
            mn, ob[:, :, 2:4], sb[:, 2:4].unsqueeze(1).to_broadcast([P, NP, 2]),
            op=mybir.AluOpType.min,
        )
        nc.vector.tensor_sub(mn, mn, mx)
        nc.vector.tensor_scalar_max(mn, mn, 0.0)
        inter = sm.tile([P, NP], F32, tag=f"inter{img}")
        nc.vector.tensor_mul(inter, mn[:, :, 0], mn[:, :, 1])
        aa2 = sm.tile([P, 2], F32, tag=f"aa2{img}")
        nc.vector.tensor_sub(aa2, sb[:, 2:4], sb[:, 0:2])
        aa = sm.tile([P, 1], F32, tag=f"aa{img}")
        nc.vector.tensor_mul(aa, aa2[:, 0:1], aa2[:, 1:2])
        # suppress iff 0.3 * union < inter  (union = area_sel + areas - inter)
        u = sm.tile([P, NP], F32, tag=f"u{img}")
        nc.vector.scalar_tensor_tensor(
            out=u, in0=ar, scalar=aa[:, 0:1], in1=inter,
            op0=mybir.AluOpType.add, op1=mybir.AluOpType.subtract,
        )
        sup = sm.tile([P, NP], mybir.dt.uint8, tag=f"sup{img}")
        nc.vector.scalar_tensor_tensor(
            out=sup, in0=u, scalar=NMS_T, in1=inter,
            op0=mybir.AluOpType.mult, op1=mybir.AluOpType.is_lt,
        )
        nc.vector.copy_predicated(sc, sup, negs)
        # kill the selected entry itself (covers zero-area self-IoU)
        nc.vector.tensor_copy(mr[:, 0:1], gm)
        nc.vector.match_replace(out=sc, in_to_replace=mr, in_values=sc,
                                imm_value=NEG)


def build_nc():
    nc = bacc.Bacc("TRN2", target_bir_lowering=False)
    rois_t = nc.dram_tensor("rois", [BPC, N, 4], F32, kind="ExternalInput")
    probs_t = nc.dram_tensor("probs", [BPC, N, C], F32, kind="ExternalInput")
    deltas_t = nc.dram_tensor("deltas", [BPC, N, C, 4], F32, kind="ExternalInput")
    out_t = nc.dram_tensor("out", [BPC, K, 6], F32, kind="ExternalOutput")
    out_flat = out_t.rearrange("b k s -> (b k s)").unsqueeze(0)  # [1, 1200]

    with TileContext(nc) as tc:
        with (
            tc.tile_pool(name="const", bufs=1) as cpool,
            tc.tile_pool(name="big", bufs=1) as big,
            tc.tile_pool(name="small", bufs=1) as sm,
            tc.tile_pool(name="psum", bufs=1, space="PSUM") as pp,
        ):
            pools = (cpool, big, sm, pp)
            state = {}

            # probs for both images: 4 free-dim chunks each (8 rois per
            # partition, 2592B descriptors), spread over the 3 DMA rings
            # (sync + scalar HWDGE, gpsimd SWDGE) so the whole stream runs
            # at the HBM rate.  img0 chunks are first in every ring's FIFO
            # so its gate accumulation starts earliest.
            NQ = NP // 4  # rois per partition per chunk (8)
            pt = []
            for img in range(BPC):
                pti = big.tile([P, NP, C], F32, tag=f"probs{img}",
                               name=f"probs{img}")
                pt.append(pti)
            state["pt"] = pt
            ap = [probs_t[img].rearrange("(p n) c -> p n c", p=P)
                  for img in range(BPC)]

            def chunk(img, c):
                sl = slice(c * NQ, (c + 1) * NQ)
                return pt[img][:, sl], ap[img][:, sl]

            ring = {0: nc.sync, 1: nc.scalar, 2: nc.gpsimd}
            # (ring, img, chunk) issue order: img0 first on each ring
            plan = [(0, 0, 0), (1, 0, 1), (2, 0, 2),
                    (0, 0, 3), (1, 1, 0), (2, 1, 1),
                    (0, 1, 2), (1, 1, 3)]
            for r, img, c in plan:
                dst, src = chunk(img, c)
                ring[r].dma_start(out=dst, in_=src)

            # zero detections; early zero-write lands under the probs stream
            # (same scalar ring as the in-guard real write => FIFO ordered)
            det = sm.tile([1, BPC * K * 6], F32, tag="det")
            nc.gpsimd.memset(det, 0.0)
            nc.scalar.dma_start(out=out_flat, in_=det)

            ones = cpool.tile([P, 1], F32, tag="ones")
            nc.gpsimd.memset(ones, 1.0)
            state["ones"] = ones
            nbias = cpool.tile([P, 1], F32, tag="nbias")
            nc.gpsimd.memset(nbias, -690000.0)

            # gate accumulation, chunk by chunk as the stream lands:
            #   vector: exact count of probs >= 0.7 for img0
            #   scalar: sum of Relu(1e6*p - 0.69e6) for img1 — exactly zero
            #     when no prob clears 0.69, strictly positive for any prob
            #     >= 0.7 (conservative over [0.69, 0.7), still exact-safe:
            #     the guard recomputes exact per-image counts)
            # chunk -> accum engine, interleaved by expected landing order so
            # neither engine is stuck with only late-landing chunks (the DVE,
            # faster per chunk, takes the stragglers)
            acc8 = sm.tile([P, 8], F32, tag="acc8")
            scrv = sm.tile([P, NQ, C], F32, tag="scrv")
            scrs = sm.tile([P, NQ, C], F32, tag="scrs")
            vec_chunks = [(0, 0), (0, 2), (1, 1), (1, 3)]
            act_chunks = [(0, 1), (0, 3), (1, 0), (1, 2)]
            for k, (img, c) in enumerate(vec_chunks):
                src, _ = chunk(img, c)
                nc.vector.tensor_scalar(
                    out=scrv, in0=src, scalar1=MIN_CONF, scalar2=None,
                    op0=mybir.AluOpType.is_ge, op1=mybir.AluOpType.add,
                    accum_out=acc8[:, k : k + 1],
                )
            for k, (img, c) in enumerate(act_chunks):
                src, _ = chunk(img, c)
                nc.scalar.activation(
                    scrs, src, mybir.ActivationFunctionType.Relu,
                    bias=nbias, scale=1e6,
                    accum_out=acc8[:, 4 + k : 5 + k],
                )

            # total gate value: >0 iff any roi might pass min-confidence
            acc1 = sm.tile([P, 1], F32, tag="acc1")
            nc.vector.reduce_sum(acc1, acc8, axis=mybir.AxisListType.X)
            totp = pp.tile([1, 1], F32, tag="totp")
            nc.tensor.matmul(totp, ones, acc1, start=True, stop=True)
            toti = sm.tile([1, 1], I32, tag="toti")
            nc.vector.tensor_copy(toti, totp)
            tot = nc.values_load(toti, min_val=0, max_val=2**30,
                                 skip_runtime_bounds_check=True)

            with tc.If(tot > 0):
                # constants used only by refine/NMS
                crev = cpool.tile([P, NP, C], F32, tag="crev")
                nc.gpsimd.iota(crev, pattern=[[0, NP], [-1, C]], base=C - 1,
                               channel_multiplier=0,
                               allow_small_or_imprecise_dtypes=True)
                negs = cpool.tile([P, NP], F32, tag="negs")
                nc.gpsimd.memset(negs, NEG)
                state["crev"] = crev
                state["negs"] = negs
                for img in range(BPC):
                    _build_image_slow(nc, tc, pools, img, rois_t, deltas_t,
                                      state, det)
                # real detections overwrite the zeros; same scalar ring as
                # the zero-write => FIFO-ordered at the destination
                nc.scalar.dma_start(out=out_flat, in_=det)
    nc.compile()
    return nc


LAST_RESULTS = None  # BassKernelResults of the most recent kernel() call


def kernel(rois, probs, deltas):
    global LAST_RESULTS
    from concourse import bass_utils

    nc = build_nc()
    in_maps = []
    for c in range(NCORES):
        sl = slice(c * BPC, (c + 1) * BPC)
        in_maps.append({
            "rois": np.ascontiguousarray(rois[sl], dtype=np.float32),
            "probs": np.ascontiguousarray(probs[sl], dtype=np.float32),
            "deltas": np.ascontiguousarray(deltas[sl], dtype=np.float32),
        })
    res = bass_utils.run_bass_kernel_spmd(nc, in_maps, core_ids=list(range(NCORES)))
    LAST_RESULTS = res
    return np.concatenate([r["out"] for r in res.results], axis=0)


if __name__ == "__main__":
    rng = np.random.default_rng(0)
    out = kernel(
        rng.random((B, N, 4), np.float32),
        rng.random((B, N, C), np.float32),
        rng.standard_normal((B, N, C, 4)).astype(np.float32),
    )
    print(out.shape, np.abs(out).max())
